# revision 2
# baseline (speedup 1.0000x reference)
"""Trainium2 Bass kernel (v10) for nn_AttentionBlock — Gram-matrix reassociated
causal attention.

Reference (per batch b):
    qs[t,j]    = sum_i s[t,i] Q[h,i,j]
    Omega[t,u] = sum_j qs[t,j] s[u,j]       (causal: keep u <= t)
    es[u,i]    = sum_j E[h,i,j] s[u,j]
    r[t,i]     = sum_h sum_u Omega[t,u] es[u,i]

Key reassociation (v10): the off-diagonal (full blocks u < bt*128) part factors
through the HEAD-INDEPENDENT Gram matrix C(bt)[j,j'] = sum_{u<bt*128} s[u,j]s[u,j']:
    r_off[t,:] = qs[t,:] @ C(bt) @ E_h^T
so per (h, bt):  M^T[j',t] = C(bt)[j,j'] qsT[j,t]  (+ diag:  s[u,j'] OmaskT[u,t])
                 rT[i,t]  += E_h^T[j',i] M^T[j',t]
C(bt) is computed ONCE (prefix-accumulated in one PSUM bank, snapshot per block),
vs the per-head es/G chain of v9 — cuts total matmul FLOPs ~20% (5.36G -> 4.28G
per core) and replaces many short-stream matmuls with N=512 streams vectorized
over all 8 heads (head dim folded into the matmul free axis).

Only diagonal 128x128 Omega^T blocks are materialized (all 8 heads in one PSUM
group), masked by one DVE multiply per 4-head half with a precomputed
triangular mask.

Distribution: data-parallel over batch (8 batches = 8 cores, no collectives).
All matmuls bf16; f32 PSUM accumulation. Output computed transposed ([n, t]) so
the final matmul needs no PE transposes; host transposes back.

PSUM budget: 5 transient 1-bank slots (qsT groups / OmegaT / M^T) + 1 bank Gram
prefix + 2 banks rT pairs = 8 banks.
"""

import numpy as np
import ml_dtypes

import concourse.bacc as bacc
import concourse.mybir as mybir
import concourse.tile as tile
from concourse.bass_utils import run_bass_kernel_spmd

B = 8      # batch (== number of cores)
T = 1024   # tokens
NF = 256   # feature dim n
H = 8      # heads
P = 128    # partitions
TB = T // P    # 8 token blocks
JC = NF // P   # 2 feature chunks
NCORES = 8

F32 = mybir.dt.float32
BF16 = mybir.dt.bfloat16
IS_GE = mybir.AluOpType.is_ge


def _emit(tc, nc, s_d, sT_d, Q_d, ET_d, out_d, ctx):
    res = ctx.enter_context(tc.tile_pool(name="res", bufs=1))
    omdp = ctx.enter_context(tc.tile_pool(name="omdp", bufs=2))
    msp = ctx.enter_context(tc.tile_pool(name="msp", bufs=2))
    pap = ctx.enter_context(tc.tile_pool(name="pap", bufs=5, space="PSUM"))
    pcp = ctx.enter_context(tc.tile_pool(name="pcp", bufs=1, space="PSUM"))
    prp = ctx.enter_context(tc.tile_pool(name="prp", bufs=2, space="PSUM"))

    s_sb = res.tile([P, TB, NF], BF16)       # [u%128, uc, j]
    sT_sb = res.tile([P, JC, T], BF16)       # [j%128, jc, t]  (also i for qsT)
    Q_sb = res.tile([P, H * JC, NF], BF16)   # [i%128, h*2+ic, j]
    ET_sb = res.tile([P, H * JC, NF], BF16)  # [j'%128, h*2+jpc, i]
    qsT = res.tile([P, JC, H, T], BF16)      # [j%128, jc, h, t]
    csnap = res.tile([P, JC, TB - 1, NF], BF16)  # [j%128, jc, k, j'] = Csum(k+1)
    mask4 = res.tile([P, 4, P], BF16)        # [u, 4, t]: 1 where u <= t
    r_out = res.tile([P, JC, T], F32)        # [i%128, ic, t]

    # Input DMAs: first-needed-first. s goes on the gpsimd queue so the Gram
    # chain can start early while the sync queue streams sT/Q.
    nc.sync.dma_start(
        out=sT_sb[:, :, 0:512],
        in_=sT_d[:, 0:512].rearrange("(c p) t -> p c t", p=P))
    nc.sync.dma_start(
        out=Q_sb[:, 0:2 * JC, :],
        in_=Q_d[0:2].rearrange("h (c p) j -> p (h c) j", p=P))
    nc.sync.dma_start(
        out=Q_sb[:, 2 * JC:4 * JC, :],
        in_=Q_d[2:4].rearrange("h (c p) j -> p (h c) j", p=P))
    nc.sync.dma_start(
        out=sT_sb[:, :, 512:],
        in_=sT_d[:, 512:].rearrange("(c p) t -> p c t", p=P))
    nc.sync.dma_start(
        out=Q_sb[:, 4 * JC:, :],
        in_=Q_d[4:].rearrange("h (c p) j -> p (h c) j", p=P))
    nc.gpsimd.dma_start(
        out=s_sb, in_=s_d.rearrange("(c p) j -> p c j", p=P))
    nc.gpsimd.dma_start(
        out=ET_sb[:, 0:4 * JC, :],
        in_=ET_d[0:4].rearrange("h (c p) j -> p (h c) j", p=P))
    nc.gpsimd.dma_start(
        out=ET_sb[:, 4 * JC:, :],
        in_=ET_d[4:].rearrange("h (c p) j -> p (h c) j", p=P))

    nc.gpsimd.memset(mask4, 1.0)
    nc.gpsimd.affine_select(
        out=mask4, in_=mask4,
        pattern=[[0, 4], [1, P]],
        compare_op=IS_GE,   # keep 1.0 where t - u >= 0, else 0
        fill=0.0, base=0, channel_multiplier=-1,
    )

    movers = [nc.vector.tensor_copy, nc.scalar.copy]
    mv = [0]

    def mover(out, in_):
        movers[mv[0] % 2](out=out, in_=in_)
        mv[0] += 1

    # ---- phase 1: qsT for all heads + Gram prefix chain ----
    Cp = pcp.tile([P, JC, NF], F32, name="Cp")

    def qs_group(h, jc, tcx):
        pw = pap.tile([P, 512], F32, tag="pw", name="pwq")
        for ic in range(JC):
            nc.tensor.matmul(
                pw,
                lhsT=Q_sb[:, h * JC + ic, jc * P:(jc + 1) * P],
                rhs=sT_sb[:, ic, tcx * 512:(tcx + 1) * 512],
                start=(ic == 0), stop=(ic == JC - 1),
                skip_group_check=True,
            )
        mover(qsT[:, jc, h, tcx * 512:(tcx + 1) * 512], pw)

    def c_link(uc):
        # Cp += s[uc]^T s[uc]; snapshot Csum(uc+1) for the bt loop
        for jc in range(JC):
            nc.tensor.matmul(
                Cp[:, jc, :],
                lhsT=s_sb[:, uc, jc * P:(jc + 1) * P],
                rhs=s_sb[:, uc, :],
                start=(uc == 0 and jc == 0), stop=(uc == TB - 2 and jc == JC - 1),
                skip_group_check=True,
            )
        mover(csnap[:, :, uc, :], Cp)

    qjobs = [(h, jc, tcx) for tcx in range(2) for h in range(H)
             for jc in range(JC)]
    k = 0
    for gi, (h, jc, tcx) in enumerate(qjobs):
        qs_group(h, jc, tcx)
        if gi % 4 == 1 and k < TB - 1:
            c_link(k)
            k += 1

    # ---- phase 2: bt loop ----
    def rt_group(pair, ms):
        pr = prp.tile([P, JC, 2 * P], F32, tag="pr", name="pr")
        n = 0
        for h in range(H):
            for jpc in range(JC):
                for ic in range(JC):
                    nc.tensor.matmul(
                        pr[:, ic, :],
                        lhsT=ET_sb[:, h * JC + jpc, ic * P:(ic + 1) * P],
                        rhs=ms[:, jpc, h, :],
                        start=(n == 0), stop=(n == H * JC * JC - 1),
                        skip_group_check=True,
                    )
                    n += 1
        for ic in range(JC):
            mover(r_out[:, ic, pair * 2 * P:(pair + 1) * 2 * P], pr[:, ic, :])
        q = nc.sync if pair % 2 == 0 else nc.scalar
        q.dma_start(
            out=out_d[:, pair * 2 * P:(pair + 1) * 2 * P].rearrange(
                "(c p) t -> p c t", p=P),
            in_=r_out[:, :, pair * 2 * P:(pair + 1) * 2 * P])

    ms_cur = None
    ms_prev = None
    for bt in range(TB):
        pair, sub = divmod(bt, 2)
        if sub == 0:
            ms_cur = msp.tile([P, JC, H, 2 * P], BF16, tag="ms",
                              name=f"ms{pair}")
        if bt >= 2 and sub == 0:
            rt_group(pair - 1, ms_prev)
        # diagonal OmegaT for all 8 heads: OmT[u, h, t] = sum_j s[u,j] qsT[j,t]
        oms = []
        for half in range(2):
            om = pap.tile([P, 4, P], F32, tag="pw", name=f"om{half}")
            for jc in range(JC):
                nc.tensor.matmul(
                    om,
                    lhsT=sT_sb[:, jc, bt * P:(bt + 1) * P],
                    rhs=qsT[:, jc, 4 * half:4 * half + 4, bt * P:(bt + 1) * P],
                    start=(jc == 0), stop=(jc == JC - 1),
                    skip_group_check=True,
                )
            oms.append(om)
        # off-diagonal via Gram: M^T[j', h, t] = Csum(bt)[j,j'] qsT[j, h, t]
        mps = []
        for jpc in range(JC):
            for half in range(2):
                mp = pap.tile([P, 4, P], F32, tag="pw", name=f"mp{jpc}{half}")
                if bt >= 1:
                    for jc in range(JC):
                        nc.tensor.matmul(
                            mp,
                            lhsT=csnap[:, jc, bt - 1, jpc * P:(jpc + 1) * P],
                            rhs=qsT[:, jc, 4 * half:4 * half + 4,
                                    bt * P:(bt + 1) * P],
                            start=(jc == 0), stop=False,
                            skip_group_check=True,
                        )
                mps.append(mp)
        # causal mask on the diagonal blocks (keep u <= t)
        omd = omdp.tile([P, H, P], BF16, tag="omd", name="omd")
        nc.vector.tensor_mul(omd[:, 0:4, :], oms[0], mask4)
        nc.vector.tensor_mul(omd[:, 4:8, :], oms[1], mask4)
        # diag contribution: M^T[j', h, t] += s[u, j'] OmaskT[u, h, t]
        for jpc in range(JC):
            for half in range(2):
                nc.tensor.matmul(
                    mps[jpc * 2 + half],
                    lhsT=s_sb[:, bt, jpc * P:(jpc + 1) * P],
                    rhs=omd[:, 4 * half:4 * half + 4, :],
                    start=(bt == 0), stop=True,
                    skip_group_check=True,
                )
        for jpc in range(JC):
            for half in range(2):
                mover(ms_cur[:, jpc, 4 * half:4 * half + 4,
                             sub * P:(sub + 1) * P],
                      mps[jpc * 2 + half])
        if sub == 1:
            ms_prev = ms_cur
    rt_group(TB // 2 - 1, ms_prev)


def build():
    from contextlib import ExitStack

    nc = bacc.Bacc(
        "TRN2",
        target_bir_lowering=False,
        debug=False,
        enable_asserts=False,
        num_devices=NCORES,
    )
    s_d = nc.dram_tensor("s", [T, NF], BF16, kind="ExternalInput").ap()
    sT_d = nc.dram_tensor("sT", [NF, T], BF16, kind="ExternalInput").ap()
    Q_d = nc.dram_tensor("Q", [H, NF, NF], BF16, kind="ExternalInput").ap()
    ET_d = nc.dram_tensor("ET", [H, NF, NF], BF16, kind="ExternalInput").ap()
    out_d = nc.dram_tensor("out", [NF, T], F32, kind="ExternalOutput").ap()
    with tile.TileContext(nc) as tc:
        with ExitStack() as ctx:
            _emit(tc, nc, s_d, sT_d, Q_d, ET_d, out_d, ctx)
    nc.compile()
    return nc


_NC = None


def _get_nc():
    global _NC
    if _NC is None:
        _NC = build()
    return _NC


def _in_maps(s, Q, E):
    bf = ml_dtypes.bfloat16
    s = np.asarray(s, dtype=np.float32)
    Qb = np.ascontiguousarray(np.asarray(Q, dtype=np.float32)).astype(bf)
    ETb = np.ascontiguousarray(
        np.asarray(E, dtype=np.float32).transpose(0, 2, 1)).astype(bf)
    return [
        {
            "s": np.ascontiguousarray(s[b]).astype(bf),
            "sT": np.ascontiguousarray(s[b].T).astype(bf),
            "Q": Qb,
            "ET": ETb,
        }
        for b in range(B)
    ]


def kernel(s, Q, E):
    nc = _get_nc()
    res = run_bass_kernel_spmd(
        nc, _in_maps(s, Q, E), core_ids=list(range(NCORES)))
    return np.stack(
        [np.ascontiguousarray(res.results[b]["out"].T) for b in range(B)],
        axis=0)


def run_profiled(s, Q, E, tmpdir=None):
    nc = _get_nc()
    res = run_bass_kernel_spmd(
        nc, _in_maps(s, Q, E), core_ids=list(range(NCORES)),
        trace=True, tmpdir=tmpdir)
    out = np.stack(
        [np.ascontiguousarray(res.results[b]["out"].T) for b in range(B)],
        axis=0)
    return out, res.exec_time_ns
